# revision 17
# baseline (speedup 1.0000x reference)
"""MeanShift retrieval-KNN loss kernel for 8 Trainium2 NeuronCores (v2).

Reference computation (B=4096, K=32768, DIM=512, TOPK=5):
    query  = l2norm(query_raw); target = l2norm(target_raw)
    qbank  = l2norm(queue); qbank[0:B] = target
    dist_t = 2 - 2 * target @ qbank.T ; dist_q = 2 - 2 * query @ qbank.T
    idx    = top5 smallest dist_t per row
    loss   = mean_b( sum_j dist_q[b, idx[b,j]] / 5 )

Sharding: queue K axis split across 8 cores (4096 bank rows each); core 0's
shard is target_raw itself (reference overwrites bank rows 0:B and raw queue
rows 0:B are never read).

v2 design, per core:
  - Host pre-transposes all operands to [128, DCH=4, N] (dim on partitions,
    dim d lives at (partition d%128, chunk d//128)), so the device does no
    PE transposes at all.
  - Normalization: squares on ACT (fp8 out), column norms via a DoubleRow
    ones-stationary matmul (broadcasts norm^2 to all 128 partitions), ACT
    sqrt + DVE reciprocal_approx_fast, scale multiply on GpSimd/DVE with a
    broadcast AP (fp8 out). The target operand is pre-scaled by SCALE=512.
  - Main loop (software-pipelined one pass deep, quadrant-blocked order):
    fp8e4 DoubleRow matmuls. Phase 1 puts 512*sim_t in PSUM; a single
    +MAGIC op rounds it (DVE passes fuse the -MAGIC in the same
    tensor_scalar; ACT passes subtract MAGIC via a tiny 2-partition bf16
    matmul so no second serial engine op sits on the PE's critical path);
    phase 2 matmuls accumulate sim_q on top (PSUM has_written bits stay
    set, so the PE adds), giving packed v = n + sim_q in PSUM. MAX8 reads
    the top-8 of each 1024-column group directly from PSUM.
  - Host merges 8 cores x 32 candidates per row, decodes
    sim_q = v - round(v), and computes the scalar loss.
"""

import numpy as np

B, K, DIM, TOPK = 4096, 32768, 512, 5
NCORES = 8
KSH = K // NCORES     # 4096 bank rows per core

P = 128               # partitions
DCH = DIM // P        # 4 dim chunks
GW = 1024             # main-loop group width (2 PSUM banks)
PCW = 512             # preproc column chunk width
SCALE = 512.0         # sim_t packing grid
MAGIC = float(3 * (2 ** 22))  # 12582912.0: +MAGIC-MAGIC rounds to int in fp32

_CACHE = {}


def build_nc(b=B, ksh=KSH, num_devices=NCORES):
    """Build + compile the per-core Bass program (identical on all cores)."""
    from contextlib import ExitStack

    import concourse.tile as tile
    from concourse import bacc, mybir

    f32 = mybir.dt.float32
    fp8 = mybir.dt.float8e4
    bf16 = mybir.dt.bfloat16
    Alu = mybir.AluOpType
    Act = mybir.ActivationFunctionType
    DR = mybir.MatmulPerfMode.DoubleRow

    NB = b // P               # batch tiles
    NSWEEP = ksh // GW        # column sweeps over the shard
    W = NSWEEP * 8            # candidates per row shipped to host
    NTQ = (b + PCW - 1) // PCW   # t/q preproc chunks
    NSH = ksh // PCW             # shard preproc chunks

    nc = bacc.Bacc(
        "TRN2", target_bir_lowering=False, debug=False, num_devices=num_devices
    )
    q_d = nc.dram_tensor("query_t", [P, DCH, b], bf16, kind="ExternalInput").ap()
    t_d = nc.dram_tensor("target_t", [P, DCH, b], bf16, kind="ExternalInput").ap()
    s_d = nc.dram_tensor("qshard_t", [P, DCH, ksh], bf16,
                         kind="ExternalInput").ap()
    o_d = nc.dram_tensor("out", [b, W], f32, kind="ExternalOutput").ap()

    with tile.TileContext(nc) as tc, ExitStack() as ctx:
        singles = ctx.enter_context(tc.tile_pool(name="singles", bufs=1))
        ld = ctx.enter_context(tc.tile_pool(name="ld", bufs=7))
        sqp = ctx.enter_context(tc.tile_pool(name="sqp", bufs=4))
        small = ctx.enter_context(tc.tile_pool(name="small", bufs=4))
        mpsum = ctx.enter_context(tc.tile_pool(name="mpsum", bufs=3, space="PSUM"))
        npsum = ctx.enter_context(tc.tile_pool(name="npsum", bufs=2, space="PSUM"))

        ones8 = singles.tile([P, 2, P], fp8)
        nc.vector.memset(ones8, 1.0)
        # -MAGIC via a 2-partition matmul: each output gets
        # colm[0,m]*1 + colm[1,m]*1 = -MAGIC accumulated into PSUM.
        colm = singles.tile([2, P], bf16)
        nc.vector.memset(colm, -MAGIC / 2)
        onesm = singles.tile([2, 512], bf16)
        nc.vector.memset(onesm, 1.0)

        # HAM warm-up: dependency-free matmuls on memset tiles keep the
        # PE busy through the idle DMA/preproc window so the clock gate
        # reaches K=8/8 before the first real matmuls issue.
        warm = mpsum.tile([P, GW], f32, tag="mm", name="warm")
        for _ in range(12):
            nc.tensor.matmul(warm[:, 0:512], colm, onesm, start=True,
                             stop=True)

        # Resident normalized fp8 operands, dim on partitions.
        sn = singles.tile([P, DCH, ksh], fp8)   # bank shard, unit rows
        tn = singles.tile([P, DCH, b], fp8)     # target * SCALE
        qn = singles.tile([P, DCH, b], fp8)     # query, unit rows
        cand = singles.tile([P, NB * W], f32)   # per-group top-8 packed values

        pre_count = [0]

        def pre_cols(src, dst, j0, cols, scaled, pfx):
            """Normalize (and optionally pre-scale) one column range."""
            cs = slice(j0, j0 + cols)
            xr = ld.tile([P, DCH, cols], bf16, tag="xr", name=f"{pfx}r")
            nc.sync.dma_start(out=xr, in_=src[:, :, cs])
            xsq = sqp.tile([P, DCH, cols], fp8, tag="sq", name=f"{pfx}s")
            nc.scalar.activation(xsq, xr, Act.Square)
            pn = npsum.tile([P, cols], f32, tag="nm", name=f"{pfx}n")
            for c in range(DCH // 2):
                nc.tensor.matmul(
                    pn, ones8, xsq[:, 2 * c:2 * c + 2, :],
                    start=(c == 0), stop=(c == DCH // 2 - 1), perf_mode=DR,
                )
            std = small.tile([P, cols], f32, tag="std", name=f"{pfx}d")
            # scaled: std = |x|/SCALE so rinv = SCALE/|x|
            nc.scalar.activation(std, pn, Act.Sqrt,
                                 scale=(1.0 / (SCALE * SCALE) if scaled else 1.0))
            rinv = small.tile([P, cols], f32, tag="rinv", name=f"{pfx}i")
            nc.vector.reciprocal_approx_fast(out=rinv, in_=std)
            rb = rinv.unsqueeze(1).broadcast_to((P, DCH, cols))
            eng = nc.vector if pre_count[0] % 3 == 2 else nc.gpsimd
            pre_count[0] += 1
            eng.tensor_tensor(out=dst[:, :, cs], in0=xr, in1=rb, op=Alu.mult)

        def pre(src, dst, j, scaled, split=False):
            ncols = dst.shape[2]
            j0 = j * PCW
            cols = min(PCW, ncols - j0)
            pfx = f"{dst.name[:2]}{j}"
            if split:
                # quarter-chunks to shorten the startup critical path
                qc = cols // 4
                for k in range(4):
                    pre_cols(src, dst, j0 + k * qc, qc, scaled, f"{pfx}_{k}")
            else:
                pre_cols(src, dst, j0, cols, scaled, pfx)

        def mm_phase(gp, lhs, s, bt, first):
            bs = slice(bt * P, (bt + 1) * P)
            for c in range(DCH // 2):
                for h in range(GW // 512):
                    ks = slice(s * GW + h * 512, s * GW + (h + 1) * 512)
                    nc.tensor.matmul(
                        gp[:, h * 512:(h + 1) * 512],
                        lhs[:, 2 * c:2 * c + 2, bs], sn[:, 2 * c:2 * c + 2, ks],
                        start=(first and c == 0), stop=(c == DCH // 2 - 1),
                        perf_mode=DR, skip_group_check=not first,
                    )

        def round_pass(gp, idx):
            # Adding MAGIC rounds SCALE*sim_t to an integer in fp32. On DVE
            # (1 in 4 passes) the -MAGIC fits in the same tensor_scalar op;
            # on ACT the -MAGIC is done by the PE (sub_magic) so no second
            # serial engine op sits on the PE's critical path. Returns
            # whether the pass still needs the PE-side subtract.
            if idx % 2 == 0:
                nc.vector.tensor_scalar(out=gp, in0=gp, scalar1=MAGIC,
                                        scalar2=-MAGIC, op0=Alu.add,
                                        op1=Alu.add)
                return False
            nc.scalar.activation(gp, gp, Act.Copy, bias=MAGIC)
            return True

        def sub_magic(gp):
            for h in range(GW // 512):
                nc.tensor.matmul(
                    gp[:, h * 512:(h + 1) * 512], colm, onesm,
                    start=False, stop=False, skip_group_check=True,
                )

        def select_pass(s, bt, gp):
            off = bt * W + s * 8
            nc.vector.max(cand[:, off:off + 8], gp)
            if s == NSWEEP - 1:
                bs = slice(bt * P, (bt + 1) * P)
                nc.gpsimd.dma_start(
                    out=o_d[bs, :], in_=cand[:, bt * W:(bt + 1) * W]
                )

        # Emission order doubles as scheduling priority. The main loop is
        # software-pipelined one pass deep: phase 1 of pass i+1 is emitted
        # before phase 2 of pass i, so the PE streams matmuls while the
        # round of pass i runs on ACT/DVE.
        #
        # Pass order is quadrant-blocked (sweep-pair x bt-half) so the
        # preproc demand for t/q chunks spreads over the first half of the
        # kernel instead of all landing in sweep 0 (GpSimd is the preproc
        # throughput limit).
        pre(s_d, sn, 0, False, split=True)
        if NSH > 1:
            pre(s_d, sn, 1, False, split=True)
        pre(t_d, tn, 0, True, split=(b >= 2 * PCW))
        pre(q_d, qn, 0, False, split=(b >= 2 * PCW))
        if NTQ > 1:
            pre(t_d, tn, 1, True)
            pre(q_d, qn, 1, False)
        for j in range(2, min(4, NSH)):
            pre(s_d, sn, j, False)

        # prefetch queue in priority order: (src, dst, chunk, scaled)
        fetch = []
        for j in range(2, NTQ):
            fetch.append((t_d, tn, j, True))
            fetch.append((q_d, qn, j, False))
        for j in range(4, NSH):
            fetch.append((s_d, sn, j, False))

        passes = []
        nbh = max(1, NB // 2)
        for sp in range(max(1, NSWEEP // 2)):
            for bh in range(0, NB, nbh):
                for s in range(2 * sp, min(2 * sp + 2, NSWEEP)):
                    for bt in range(bh, min(bh + nbh, NB)):
                        passes.append((s, bt))

        done = {(sn.name, 0), (tn.name, 0), (qn.name, 0)}
        for j in range(1, min(4, NSH)):
            done.add((sn.name, j))
        if NTQ > 1:
            done.add((tn.name, 1))
            done.add((qn.name, 1))

        def emit_fetch(item):
            src, dst, j, scaled = item
            if (dst.name, j) not in done:
                pre(src, dst, j, scaled)
                done.add((dst.name, j))

        def need(s, bt):
            req = [(s_d, sn, j, False)
                   for j in range(s * GW // PCW, ((s + 1) * GW - 1) // PCW + 1)]
            req.append((t_d, tn, bt * P // PCW, True))
            req.append((q_d, qn, bt * P // PCW, False))
            for item in req:
                emit_fetch(item)

        fi = 0
        prev = None
        for idx, (s, bt) in enumerate(passes):
            need(s, bt)
            gp = mpsum.tile([P, GW], f32, tag="mm", name=f"g{s}_{bt}")
            mm_phase(gp, tn, s, bt, True)
            sub = round_pass(gp, idx)
            if prev is not None:
                pgp, ps, pbt, psub = prev
                if psub:
                    sub_magic(pgp)
                mm_phase(pgp, qn, ps, pbt, False)
                select_pass(ps, pbt, pgp)
            prev = (gp, s, bt, sub)
            if idx % 2 == 0 and fi < len(fetch):
                emit_fetch(fetch[fi])
                fi += 1
        gp, s, bt, sub = prev
        if sub:
            sub_magic(gp)
        mm_phase(gp, qn, s, bt, False)
        select_pass(s, bt, gp)
        while fi < len(fetch):
            emit_fetch(fetch[fi])
            fi += 1

    nc.compile()
    return nc


def _get_nc():
    key = (B, KSH, NCORES)
    if key not in _CACHE:
        _CACHE[key] = build_nc()
    return _CACHE[key]


def prep_t(x):
    """[N, 512] f32 -> [128, 4, N] bf16: out[p, dc, n] = x[n, dc*128 + p]."""
    import ml_dtypes
    n = x.shape[0]
    return np.ascontiguousarray(
        x.T.reshape(DCH, P, n).transpose(1, 0, 2).astype(ml_dtypes.bfloat16)
    )


def merge_host(cand_v, topk=TOPK):
    """cand_v: [ncores, b, W] packed values -> scalar loss (float32)."""
    b = cand_v.shape[1]
    allv = np.transpose(cand_v, (1, 0, 2)).reshape(b, -1)
    part = np.partition(allv, allv.shape[1] - topk, axis=1)[:, -topk:]
    sim_q = part - np.round(part)
    dist_q = 2.0 - 2.0 * sim_q
    return np.float32(dist_q.mean())


def run_device(query_raw, target_raw, queue, **spmd_kwargs):
    """Run the 8-core SPMD program; returns (loss, BassKernelResults)."""
    from concourse.bass_utils import run_bass_kernel_spmd

    q = prep_t(np.asarray(query_raw, dtype=np.float32))
    t = prep_t(np.asarray(target_raw, dtype=np.float32))
    qu = np.asarray(queue, dtype=np.float32)

    nc = _get_nc()
    in_maps = []
    for c in range(NCORES):
        shard = t if c == 0 else prep_t(qu[c * KSH:(c + 1) * KSH])
        in_maps.append({"query_t": q, "target_t": t, "qshard_t": shard})
    bres = run_bass_kernel_spmd(nc, in_maps, list(range(NCORES)), **spmd_kwargs)
    cand = np.stack([bres.results[c]["out"] for c in range(NCORES)], axis=0)
    return merge_host(cand), bres


def kernel(query_raw, target_raw, queue):
    loss, _ = run_device(query_raw, target_raw, queue)
    return loss
